# revision 1
# baseline (speedup 1.0000x reference)
"""DigitCaps dynamic-routing kernel for 8 TRN2 NeuronCores.

Math refactor (u_hat is NEVER materialized - it would be 189 MB):
  u_hat[b,r,c,d] = sum_i W[r,c,d,i] * u[b,r,i]
  softmax over r without max-subtraction (b_ij values are O(1)):
      c_ij[r,c,d] = exp(b[r,c,d]) / Z[c,d],  Z = sum_r exp(b)
  s[b,c,d]  = (sum_{r,i} (exp(b) * W)[r,c,d,i] u[b,r,i]) / Z[c,d]
  v = squash(s)
  b += (1/B) sum_b t[b,r,c] v[b,c,d],  t[b,r,c] = sum_i (sum_d W) u[b,r,i]
       (t is iteration-invariant -> computed once)

Sharding: routes (R=1152) split across 8 cores (144 each). Per iteration one
fused AllReduce carries the partial s' (B x C*D) and partial Z. Everything
else is local; v is computed redundantly on every core; core 0's output is
returned.

Per-core layout: contraction dim K = (r_local, i) = 1152 = 9 chunks of 128.
All matmuls put K on partitions; host pre-permutes u and W into that layout
(pure permutation - no reference compute happens on host).
"""

import os
import numpy as np

B, R, C, D, I = 256, 1152, 10, 16, 8
CD = C * D                 # 160
NCORES = 8
RL = R // NCORES           # 144 routes per core
NCHUNK = RL * I // 128     # 9 K-chunks of 128
NG = 3                     # chunk groups of 3 (b_ij tile partition packing)
NITER = 3
EPS = 1e-5

_CACHE = {}


def _build_program():
    from contextlib import ExitStack

    import concourse.bass as bass
    import concourse.bacc as bacc
    import concourse.mybir as mybir
    import concourse.tile as tile

    f32 = mybir.dt.float32
    AF = mybir.ActivationFunctionType

    nc = bacc.Bacc(None, num_devices=NCORES)

    # One fused input parameter -> one DMA -> one DMA semaphore, so no PE
    # instruction ever needs two sync waits (codegen limit on S3_LW).
    # Layout: [0:2304] uT | [2304:3744] Wt | [3744:3904] mask | [3904:4288] expand
    DW = NCHUNK * B + NCHUNK * CD + CD + NG * 128
    data_d = nc.declare_dram_parameter("data", [128, DW], f32, isOutput=False)
    out_d = nc.declare_dram_parameter("out", [B, CD], f32, isOutput=True)

    rgroups = [list(range(NCORES))]

    with tile.TileContext(nc) as tc, ExitStack() as ctx:
        singles = ctx.enter_context(tc.tile_pool(name="singles", bufs=1))
        wcpool = ctx.enter_context(tc.tile_pool(name="wc", bufs=3))
        stpool = ctx.enter_context(tc.tile_pool(name="stage", bufs=2))
        work = ctx.enter_context(tc.tile_pool(name="work", bufs=8))
        ps_s = ctx.enter_context(tc.tile_pool(name="ps_s", bufs=1, space="PSUM"))
        ps_e = ctx.enter_context(tc.tile_pool(name="ps_e", bufs=2, space="PSUM"))
        ps_z = ctx.enter_context(tc.tile_pool(name="ps_z", bufs=1, space="PSUM"))
        ps_b = ctx.enter_context(tc.tile_pool(name="ps_b", bufs=2, space="PSUM"))
        dram = ctx.enter_context(tc.tile_pool(name="dram", bufs=1, space="DRAM"))

        cc = []
        for it in range(NITER):
            w = 2 * CD if it == 0 else 3 * CD
            ci = dram.tile([128, w], f32, tag=f"cc_in{it}", name=f"cc_in{it}")
            co = dram.tile([128, w], f32, tag=f"cc_out{it}", name=f"cc_out{it}")
            cc.append((ci, co, w))

        sb_data = singles.tile([128, DW], f32, tag="data")
        nc.sync.dma_start(out=sb_data, in_=data_d[:])
        o_uT, o_Wt = 0, NCHUNK * B
        o_mk, o_ex = o_Wt + NCHUNK * CD, o_Wt + NCHUNK * CD + CD
        sb_uT = sb_data[:, o_uT:o_uT + NCHUNK * B]
        sb_Wt = sb_data[:, o_Wt:o_Wt + NCHUNK * CD]
        sb_mask = sb_data[:, o_mk:o_mk + CD]
        sb_ex = sb_data[0:48, o_ex:o_ex + NG * 128]

        bf16 = mybir.dt.bfloat16
        # bf16 copies of all matmul operands (PE runs ~4x faster than fp32r)
        sb_uTb = singles.tile([128, NCHUNK * B], bf16, tag="uTb")
        nc.vector.tensor_copy(out=sb_uTb, in_=sb_uT)
        sb_Wtb = singles.tile([128, NCHUNK * CD], bf16, tag="Wtb")
        nc.vector.tensor_copy(out=sb_Wtb, in_=sb_Wt)
        sb_exb = singles.tile([48, NG * 128], bf16, tag="exb")
        nc.vector.tensor_copy(out=sb_exb, in_=sb_ex)

        sb_ones = singles.tile([48, 128], bf16, tag="ones")
        nc.vector.memset(sb_ones, 1.0)

        # Wd[(rp,i), (k,c)] = (1/B) * sum_d Wt   (t pre-scaled by 1/B here)
        sb_Wd = singles.tile([128, NCHUNK * C], f32, tag="Wd")
        for k in range(NCHUNK):
            nc.vector.reduce_sum(
                out=sb_Wd[:, k * C:(k + 1) * C],
                in_=sb_Wt[:, k * CD:(k + 1) * CD].rearrange("p (c d) -> p c d", d=D),
                axis=mybir.AxisListType.X,
            )
        nc.vector.tensor_scalar_mul(sb_Wd, sb_Wd, 1.0 / B)

        # Block-diagonal Wd for the t matmul, built in one full-partition op:
        # Wdbd[p, k*CD + rp*C + c] = Wd[p,(k,c)] * mask[p, rp*C + c]
        # where mask[p, rp*C+c] = (rp == p//8). Zero-stride APs broadcast
        # Wd over rp and the mask over k.
        sb_Wdbd = singles.tile([128, NCHUNK * CD], bf16, tag="Wdbd")
        wd_b = bass.AP(
            tensor=sb_Wd.tensor, offset=sb_Wd.offset,
            ap=[sb_Wd.ap[0], [C, NCHUNK], [0, 16], [1, C]],
        )
        mk_b = bass.AP(
            tensor=sb_mask.tensor, offset=sb_mask.offset,
            ap=[sb_mask.ap[0], [0, NCHUNK], [C, 16], [1, C]],
        )
        nc.vector.tensor_mul(
            sb_Wdbd.rearrange("p (k rp c) -> p k rp c", rp=16, c=C), wd_b, mk_b
        )

        # t[b, (k, rp, c)] = sum_i Wd[(rp,i),(k,c)] u[b, r(k,rp), i]
        sb_t = [singles.tile([128, NCHUNK * CD], bf16, tag=f"t{bh}", name=f"t{bh}") for bh in range(2)]
        for k in range(NCHUNK):
            for bh in range(2):
                pt = ps_e.tile([128, CD], f32, tag="pe", name="pt")
                nc.tensor.matmul(
                    pt,
                    sb_uTb[:, k * B + bh * 128: k * B + (bh + 1) * 128],
                    sb_Wdbd[:, k * CD:(k + 1) * CD],
                    start=True, stop=True,
                )
                nc.vector.tensor_copy(out=sb_t[bh][:, k * CD:(k + 1) * CD], in_=pt)

        # b_ij tile: partitions (j, rp) with j = chunk % 3, free (g, c, d)
        sb_b = singles.tile([48, NG * CD], f32, tag="b")
        nc.vector.memset(sb_b, 0.0)
        sb_E = singles.tile([48, NG * CD], bf16, tag="E")
        sb_vf = singles.tile([128, 2 * CD], f32, tag="vf")
        sb_vb = singles.tile([128, 2 * CD], bf16, tag="vb")

        for it in range(NITER):
            ci, co, w = cc[it]
            if it > 0:
                nc.scalar.activation(out=sb_E, in_=sb_b, func=AF.Exp)
                pz = ps_z.tile([128, NG * CD], f32, tag="pz")
                nc.tensor.matmul(pz, sb_ones, sb_E, start=True, stop=True)

            # s'[bh][b, cd] = sum_k sum_K uT_k[K, b] * (Wt_k * E_k)[K, cd]
            st = [ps_s.tile([128, CD], f32, tag=f"s{bh}", name=f"s{bh}") for bh in range(2)]
            for k in range(NCHUNK):
                if it > 0:
                    g, j = k // NG, k % NG
                    pe = ps_e.tile([128, CD], f32, tag="pe")
                    nc.tensor.matmul(
                        pe,
                        sb_exb[:, j * 128:(j + 1) * 128],
                        sb_E[:, g * CD:(g + 1) * CD],
                        start=True, stop=True,
                    )
                    rhs = wcpool.tile([128, CD], bf16, tag="wc")
                    nc.vector.tensor_mul(rhs, sb_Wt[:, k * CD:(k + 1) * CD], pe)
                else:
                    rhs = sb_Wtb[:, k * CD:(k + 1) * CD]
                for bh in range(2):
                    nc.tensor.matmul(
                        st[bh],
                        sb_uTb[:, k * B + bh * 128: k * B + (bh + 1) * 128],
                        rhs,
                        start=(k == 0), stop=(k == NCHUNK - 1),
                    )

            # Stage partials and AllReduce (s' || Z in one collective)
            stage = stpool.tile([128, w], f32, tag="stage")
            for bh in range(2):
                nc.vector.tensor_copy(out=stage[:, bh * CD:(bh + 1) * CD], in_=st[bh])
            if it > 0:
                # Z[cd] = sum_g pz[:, g*CD+cd] - strided reduce, one PSUM read
                pz_t = bass.AP(
                    tensor=pz.tensor, offset=pz.offset,
                    ap=[pz.ap[0], [1, CD], [CD, NG]],
                )
                nc.vector.reduce_sum(
                    out=stage[:, 2 * CD:3 * CD], in_=pz_t,
                    axis=mybir.AxisListType.X,
                )
            nc.sync.dma_start(out=ci[:], in_=stage)
            nc.gpsimd.collective_compute(
                "AllReduce", mybir.AluOpType.add,
                replica_groups=rgroups, ins=[ci.opt()], outs=[co.opt()],
            )
            red = stpool.tile([128, w], f32, tag="red")
            nc.sync.dma_start(out=red, in_=co[:])

            # v = squash(s_sum / Z), both b-halves fused into (128, 2*CD) ops
            x = work.tile([128, 2 * CD], f32, tag="x")
            if it == 0:
                nc.vector.tensor_scalar_mul(x, red[:, 0:2 * CD], 1.0 / R)
            else:
                rz = work.tile([128, CD], f32, tag="rz")
                nc.vector.reciprocal(rz, red[:, 2 * CD:3 * CD])
                rz2 = bass.AP(tensor=rz.tensor, offset=rz.offset,
                              ap=[rz.ap[0], [0, 2], [1, CD]])
                nc.vector.tensor_mul(
                    x.rearrange("p (h f) -> p h f", f=CD),
                    red[:, 0:2 * CD].rearrange("p (h f) -> p h f", f=CD),
                    rz2,
                )
            sq = work.tile([128, 2 * CD], f32, tag="sq")
            nc.vector.tensor_mul(sq, x, x)
            den = work.tile([128, 2 * CD], f32, tag="den")
            nc.scalar.activation(out=den, in_=sq, func=AF.Sqrt)
            nc.vector.tensor_scalar_add(den, den, EPS)
            den2 = work.tile([128, 2 * CD], f32, tag="den2")
            nc.vector.tensor_scalar_add(den2, sq, 1.0)
            nc.vector.tensor_mul(den, den, den2)
            nc.vector.reciprocal(den, den)
            nc.vector.tensor_mul(sq, sq, den)
            nc.vector.tensor_mul(sb_vf, x, sq)
            if it < NITER - 1:
                nc.vector.tensor_copy(out=sb_vb, in_=sb_vf)

            if it < NITER - 1:
                # b[(j,rp), g*CD + c*D + d] += sum_b t[b,(g*3+j),rp,c] v[b, c*D+d]
                t_r = [sb_t[bh].rearrange("p (k rp c) -> p k rp c", rp=16, c=C)
                       for bh in range(2)]
                for g in range(NG):
                    pb = ps_b.tile([48, CD], f32, tag="pb")
                    for c in range(C):
                        for bh in range(2):
                            nc.tensor.matmul(
                                pb[:, c * D:(c + 1) * D],
                                t_r[bh][:, g * NG:(g + 1) * NG, :, c],
                                sb_vb[:, bh * CD + c * D:bh * CD + (c + 1) * D],
                                start=(bh == 0), stop=(bh == 1),
                            )
                    nc.vector.tensor_add(
                        sb_b[:, g * CD:(g + 1) * CD],
                        sb_b[:, g * CD:(g + 1) * CD],
                        pb,
                    )
            else:
                for bh in range(2):
                    nc.sync.dma_start(
                        out=out_d[bh * 128:(bh + 1) * 128, :],
                        in_=sb_vf[:, bh * CD:(bh + 1) * CD],
                    )

    nc.compile()
    return nc


def _host_inputs(u, W):
    """Pure-permutation host prep: per-core (r,i)-major layouts."""
    u = np.ascontiguousarray(u, dtype=np.float32)
    W = np.ascontiguousarray(W, dtype=np.float32)
    expand = np.zeros((48, NG * 128), dtype=np.float32)
    for j in range(NG):
        for p in range(128):
            expand[16 * j + p // 8, j * 128 + p] = 1.0
    mask = np.zeros((128, CD), dtype=np.float32)
    for p in range(128):
        mask[p, (p // 8) * C:(p // 8) * C + C] = 1.0
    DW = NCHUNK * B + NCHUNK * CD + CD + NG * 128
    o_uT, o_Wt = 0, NCHUNK * B
    o_mk, o_ex = o_Wt + NCHUNK * CD, o_Wt + NCHUNK * CD + CD
    in_maps = []
    for ci in range(NCORES):
        rs = ci * RL
        usl = u[:, rs:rs + RL, :].reshape(B, RL * I).T          # (1152, 256)
        uTd = usl.reshape(NCHUNK, 128, B).transpose(1, 0, 2).reshape(128, NCHUNK * B)
        wsl = W[rs:rs + RL].transpose(0, 3, 1, 2).reshape(RL * I, CD)
        Wtd = wsl.reshape(NCHUNK, 128, CD).transpose(1, 0, 2).reshape(128, NCHUNK * CD)
        data = np.zeros((128, DW), dtype=np.float32)
        data[:, o_uT:o_uT + NCHUNK * B] = uTd
        data[:, o_Wt:o_Wt + NCHUNK * CD] = Wtd
        data[:, o_mk:o_mk + CD] = mask
        data[:48, o_ex:o_ex + NG * 128] = expand
        in_maps.append({"data": data})
    return in_maps


def _install_profile_hook():
    """Recreate the missing antenv.axon_hooks NTFF-profile hook (dev only)."""
    import contextlib
    import ctypes
    import sys
    import types

    try:
        from antenv.axon_hooks import get_axon_ntff_profile_hook  # noqa: F401
        return
    except ImportError:
        pass

    mod = types.ModuleType("antenv.axon_hooks")
    holder = {}
    mod.set_axon_ntff_profile_hook = lambda h: holder.__setitem__("h", h)
    mod.get_axon_ntff_profile_hook = lambda: holder.get("h")
    import antenv

    sys.modules["antenv.axon_hooks"] = mod
    antenv.axon_hooks = mod

    so_path = "/opt/axon/libaxon_pjrt.so"
    lib = ctypes.CDLL(so_path)
    if not hasattr(lib, "axon_start_nrt_profile"):
        return
    lib.axon_start_nrt_profile.argtypes = [
        ctypes.POINTER(ctypes.c_int64),
        ctypes.c_size_t,
    ]
    lib.axon_start_nrt_profile.restype = ctypes.c_int64
    lib.axon_stop_nrt_profile.argtypes = [ctypes.c_char_p]
    lib.axon_stop_nrt_profile.restype = ctypes.c_int64

    @contextlib.contextmanager
    def _hook(output_dir, device_ids):
        import jax

        jax.devices()
        if device_ids:
            ids = (ctypes.c_int64 * len(device_ids))(*device_ids)
            rc = lib.axon_start_nrt_profile(ids, len(device_ids))
        else:
            rc = lib.axon_start_nrt_profile(None, 0)
        if rc != 0:
            raise RuntimeError(f"axon_start_nrt_profile rc={rc}")
        try:
            yield
        finally:
            n = lib.axon_stop_nrt_profile(str(output_dir).encode())
            print(f"profile: {n} file(s) written to {output_dir}")

    mod.set_axon_ntff_profile_hook(_hook)

    # Avoid the bucket upload inside the trace post-processing.
    import concourse.bass_utils as bu

    bu.upload_artifacts = lambda tmpdir: f"local:{tmpdir}"


def kernel(u, W):
    from concourse.bass_utils import run_bass_kernel_spmd

    if os.environ.get("KERNEL_TRACE", "0") == "1":
        _install_profile_hook()
    if "nc" not in _CACHE:
        _CACHE["nc"] = _build_program()
    nc = _CACHE["nc"]
    in_maps = _host_inputs(u, W)
    trace = os.environ.get("KERNEL_TRACE", "0") == "1"
    res = run_bass_kernel_spmd(
        nc, in_maps, core_ids=list(range(NCORES)), trace=trace
    )
    _CACHE["last_result"] = res
    return np.asarray(res.results[0]["out"]).reshape(B, C, D)

